# revision 5
# baseline (speedup 1.0000x reference)
"""Local-window attention encoder layer on 8 Trainium2 cores.

Problem: B=4, S=8192, D=512, window W=128, H=8 heads (HD=64), FF dim 2048.
Sharding: [B*nW]=256 independent windows split 32/core across 8 cores.

Per-core device kernel (bf16 matmuls, f32 residual/softmax/LN):
  windows processed in pairs (256 tokens on the matmul free axis).
  qkT (e-major) via W-stationary matmuls; v kept token-major; scores per
  head from qkT slices; softmax over the free axis with ACT-accumulated
  row sums; probs normalized then PE-transposed for attnT; out-proj /
  FF1 / FF2 as K-chunked accumulating matmuls with rank-1 bias matmuls.
"""

import numpy as np
import ml_dtypes

import concourse.bass as bass
import concourse.tile as tile
from concourse import bacc, mybir
from concourse.bass_utils import run_bass_kernel_spmd

BF16 = ml_dtypes.bfloat16
F32 = mybir.dt.float32
BF = mybir.dt.bfloat16
AF = mybir.ActivationFunctionType
ALU = mybir.AluOpType

D = 512
H = 8
W = 128
HD = 64
FF = 2048
EPS = 1e-5
N_CORES = 8
B, S = 4, 8192
NW_TOT = (B * S) // W          # 256 windows
WPC = NW_TOT // N_CORES        # 32 windows per core
PAIRS = WPC // 2               # 16 pairs per core
KC = D // 128                  # 4 contraction chunks of 128
FC = FF // 128                 # 16 ff chunks


def _build_nc(n_pairs=PAIRS):
    nc = bacc.Bacc("TRN2", target_bir_lowering=False, debug=False,
                   num_devices=N_CORES)
    n_tok = n_pairs * 2 * W

    x_d = nc.dram_tensor("x", [n_tok, D], F32, kind="ExternalInput").ap()
    out_d = nc.dram_tensor("out", [n_tok, D], F32, kind="ExternalOutput").ap()
    wqk_d = nc.dram_tensor("wqk", [128, KC * 1024], BF, kind="ExternalInput").ap()
    wv_d = nc.dram_tensor("wv", [128, KC * D], BF, kind="ExternalInput").ap()
    wo_d = nc.dram_tensor("wo", [128, KC * D], BF, kind="ExternalInput").ap()
    w1_d = nc.dram_tensor("w1t", [128, KC * FF], BF, kind="ExternalInput").ap()
    w2_d = nc.dram_tensor("w2t", [128, FC * D], BF, kind="ExternalInput").ap()
    qkb_d = nc.dram_tensor("qkb", [128, 8], F32, kind="ExternalInput").ap()
    b1_d = nc.dram_tensor("b1t", [128, FC], F32, kind="ExternalInput").ap()
    vb_d = nc.dram_tensor("vbr", [1, D], BF, kind="ExternalInput").ap()
    ob_d = nc.dram_tensor("obr", [1, D], BF, kind="ExternalInput").ap()
    b2_d = nc.dram_tensor("b2r", [1, D], BF, kind="ExternalInput").ap()
    g1_d = nc.dram_tensor("g1b", [128, D], F32, kind="ExternalInput").ap()
    bb1_d = nc.dram_tensor("bb1", [128, D], F32, kind="ExternalInput").ap()
    g2_d = nc.dram_tensor("g2b", [128, D], F32, kind="ExternalInput").ap()
    bb2_d = nc.dram_tensor("bb2", [128, D], F32, kind="ExternalInput").ap()
    id_d = nc.dram_tensor("ident", [128, 128], BF, kind="ExternalInput").ap()
    on_d = nc.dram_tensor("ones1", [1, 128], BF, kind="ExternalInput").ap()

    xv = x_d.rearrange("(w p) d -> w p d", p=W)
    ov = out_d.rearrange("(w p) d -> w p d", p=W)

    with tile.TileContext(nc) as tc:
        with (
            tc.tile_pool(name="const", bufs=1) as cp,
            tc.tile_pool(name="stream", bufs=2) as sp,
            tc.tile_pool(name="deep", bufs=4) as sp4,
            tc.tile_pool(name="ps", bufs=3, space="PSUM") as pp,
            tc.tile_pool(name="pst", bufs=2, space="PSUM") as ppt,
            tc.tile_pool(name="sc", bufs=3, space="PSUM") as scp,
        ):
            # ---- resident constants ----
            wqk = cp.tile([128, KC, 1024], BF); nc.sync.dma_start(wqk[:], wqk_d[:])
            wv = cp.tile([128, KC, D], BF); nc.sync.dma_start(wv[:], wv_d[:])
            wo = cp.tile([128, KC, D], BF); nc.sync.dma_start(wo[:], wo_d[:])
            w1t = cp.tile([128, KC, FF], BF); nc.sync.dma_start(w1t[:], w1_d[:])
            w2t = cp.tile([128, FC, D], BF); nc.sync.dma_start(w2t[:], w2_d[:])
            qkb = cp.tile([128, 8], F32); nc.sync.dma_start(qkb[:], qkb_d[:])
            b1t = cp.tile([128, FC], F32); nc.sync.dma_start(b1t[:], b1_d[:])
            vbr = cp.tile([1, D], BF); nc.sync.dma_start(vbr[:], vb_d[:])
            obr = cp.tile([1, D], BF); nc.sync.dma_start(obr[:], ob_d[:])
            b2r = cp.tile([1, D], BF); nc.sync.dma_start(b2r[:], b2_d[:])
            g1b = cp.tile([128, D], F32); nc.sync.dma_start(g1b[:], g1_d[:])
            bb1 = cp.tile([128, D], F32); nc.sync.dma_start(bb1[:], bb1_d[:])
            g2b = cp.tile([128, D], F32); nc.sync.dma_start(g2b[:], g2_d[:])
            bb2 = cp.tile([128, D], F32); nc.sync.dma_start(bb2[:], bb2_d[:])
            ident = cp.tile([128, 128], BF); nc.sync.dma_start(ident[:], id_d[:])
            eps_t = cp.tile([128, 1], F32); nc.vector.memset(eps_t[:], EPS)
            ones1 = cp.tile([1, 128], BF); nc.sync.dma_start(ones1[:], on_d[:])

            def layernorm(y, g, b, out_f32, out_bf16=None):
                """y [128,D] f32 (already residual-added, row-sum in st col0).
                Returns nothing; writes out tiles. st supplied by caller."""
                pass  # inlined below instead

            for p in range(n_pairs):
                xw = []
                for w in range(2):
                    x = sp4.tile([W, D], F32, tag="x")
                    nc.sync.dma_start(x[:], xv[2 * p + w])
                    xw.append(x)

                # transpose x (bf16) -> xT_pair [128, KC, 256]
                xtp = sp.tile([128, KC, 2 * W], BF, tag="xtp")
                for w in range(2):
                    xb = sp.tile([W, D], BF, tag="xb")
                    nc.vector.tensor_copy(xb[:], xw[w][:])
                    tp = ppt.tile([128, KC, 128], BF, tag="pst")
                    for k in range(KC):
                        nc.tensor.transpose(tp[:, k, :], xb[:, k * 128:(k + 1) * 128],
                                            ident[:])
                    nc.scalar.copy(xtp[:, :, w * W:(w + 1) * W], tp[:])

                # qkT: [e-chunk 8][128, 256]
                qkt = sp.tile([128, 8, 2 * W], BF, tag="qkt")
                for m in range(8):
                    pq = pp.tile([128, 2 * W], F32, tag="ps")
                    for k in range(KC):
                        nc.tensor.matmul(
                            pq[:], wqk[:, k, m * 128:(m + 1) * 128], xtp[:, k, :],
                            start=(k == 0), stop=(k == KC - 1))
                    nc.scalar.activation(qkt[:, m, :], pq[:], AF.Identity,
                                         bias=qkb[:, m:m + 1])

                # v token-major per window: [128 tok, 512]
                vw = []
                for w in range(2):
                    pv = pp.tile([128, D], F32, tag="ps")
                    for k in range(KC):
                        nc.tensor.matmul(
                            pv[:], xtp[:, k, w * W:(w + 1) * W], wv[:, k, :],
                            start=(k == 0), stop=False)
                    nc.tensor.matmul(pv[:], ones1[:], vbr[:], start=False, stop=True)
                    v = sp4.tile([128, D], BF, tag="v")
                    nc.vector.tensor_copy(v[:], pv[:])
                    vw.append(v)

                ln1fw = []
                for w in range(2):
                    # ---- attention ----
                    sums = sp.tile([128, 8], F32, tag="sums")
                    probs = []
                    for h in range(8):
                        psc = scp.tile([128, 128], F32, tag="sc")
                        pb = (h % 2) * 64
                        lq = qkt[pb:pb + 64, h // 2, w * W:(w + 1) * W]
                        lk = qkt[pb:pb + 64, 4 + h // 2, w * W:(w + 1) * W]
                        nc.tensor.matmul(psc[:], lq, lk, start=True, stop=True,
                                         tile_position=(pb, 0))
                        pr = sp.tile([128, 128], BF, tag=f"pr{h}")
                        nc.scalar.activation(pr[:], psc[:], AF.Exp,
                                             accum_out=sums[:, h:h + 1])
                        probs.append(pr)
                    recip = sp.tile([128, 8], F32, tag="recip")
                    nc.vector.reciprocal(recip[:], sums[:])
                    pat = pp.tile([128, D], F32, tag="ps")
                    for h in range(8):
                        pn = sp.tile([128, 128], BF, tag=f"pn{h%4}")
                        nc.vector.tensor_scalar_mul(pn[:], probs[h][:],
                                                    recip[:, h:h + 1])
                        ptp = scp.tile([128, 128], BF, tag="sc")
                        nc.tensor.transpose(ptp[:], pn[:], ident[:])
                        pts = sp.tile([128, 128], BF, tag=f"pt{h%4}")
                        nc.scalar.copy(pts[:], ptp[:])
                        pb = (h % 2) * 64
                        nc.tensor.matmul(
                            pat[pb:pb + 64, (h // 2) * 128:(h // 2 + 1) * 128],
                            vw[w][:, h * HD:(h + 1) * HD], pts[:],
                            start=True, stop=True, tile_position=(0, pb))
                    ats = sp.tile([128, D], BF, tag="ats")
                    nc.scalar.copy(ats[:], pat[:])

                    # out projection
                    pao = pp.tile([128, D], F32, tag="ps")
                    for k in range(KC):
                        nc.tensor.matmul(pao[:], ats[:, k * 128:(k + 1) * 128],
                                         wo[:, k, :], start=(k == 0), stop=False)
                    nc.tensor.matmul(pao[:], ones1[:], obr[:], start=False, stop=True)

                    # residual 1 + LN1
                    st = sp.tile([128, 8], F32, tag="st")
                    y1 = sp.tile([128, D], F32, tag="y1")
                    nc.vector.scalar_tensor_tensor(y1[:], pao[:], 0.0, xw[w][:],
                                                   ALU.add, ALU.add,
                                                   accum_out=st[:, 0:1])
                    sq = sp.tile([128, D], F32, tag="sq")
                    nc.vector.scalar_tensor_tensor(sq[:], y1[:], 0.0, y1[:],
                                                   ALU.add, ALU.mult,
                                                   accum_out=st[:, 1:2])
                    nc.scalar.mul(st[:, 2:3], st[:, 0:1], 1.0 / D)
                    nc.vector.tensor_mul(st[:, 3:4], st[:, 2:3], st[:, 2:3])
                    nc.vector.scalar_tensor_tensor(st[:, 4:5], st[:, 1:2], 1.0 / D,
                                                   st[:, 3:4], ALU.mult, ALU.subtract)
                    nc.scalar.activation(st[:, 5:6], st[:, 4:5], AF.Sqrt, bias=eps_t[:])
                    nc.vector.reciprocal(st[:, 6:7], st[:, 5:6])
                    nc.vector.scalar_tensor_tensor(st[:, 7:8], st[:, 2:3], -1.0,
                                                   st[:, 6:7], ALU.mult, ALU.mult)
                    xh = sp.tile([128, D], F32, tag="xh")
                    nc.scalar.activation(xh[:], y1[:], AF.Identity,
                                         bias=st[:, 7:8], scale=st[:, 6:7])
                    ln1f = sp4.tile([128, D], F32, tag="ln1f")
                    nc.vector.tensor_mul(xh[:], xh[:], g1b[:])
                    nc.vector.tensor_add(ln1f[:], xh[:], bb1[:])
                    ln1fw.append(ln1f)

                # transpose ln1 (both windows) -> lnT [128, KC, 256]
                lnt = sp.tile([128, KC, 2 * W], BF, tag="lnt")
                for w in range(2):
                    lb = sp.tile([W, D], BF, tag="lb")
                    nc.vector.tensor_copy(lb[:], ln1fw[w][:])
                    tp2 = ppt.tile([128, KC, 128], BF, tag="pst")
                    for k in range(KC):
                        nc.tensor.transpose(tp2[:, k, :], lb[:, k * 128:(k + 1) * 128],
                                            ident[:])
                    nc.scalar.copy(lnt[:, :, w * W:(w + 1) * W], tp2[:])

                # FF1: h1T [f-chunk 16][128, 256]
                h1 = sp.tile([128, FC, 2 * W], BF, tag="h1")
                for m in range(FC):
                    ph = pp.tile([128, 2 * W], F32, tag="ps")
                    for k in range(KC):
                        nc.tensor.matmul(
                            ph[:], w1t[:, k, m * 128:(m + 1) * 128], lnt[:, k, :],
                            start=(k == 0), stop=(k == KC - 1))
                    nc.scalar.activation(h1[:, m, :], ph[:], AF.Relu,
                                         bias=b1t[:, m:m + 1])

                # FF2 + residual2 + LN2 per window
                for w in range(2):
                    pf = pp.tile([128, D], F32, tag="ps")
                    for m in range(FC):
                        nc.tensor.matmul(pf[:], h1[:, m, w * W:(w + 1) * W],
                                         w2t[:, m, :], start=(m == 0), stop=False)
                    nc.tensor.matmul(pf[:], ones1[:], b2r[:], start=False, stop=True)

                    st2 = sp.tile([128, 8], F32, tag="st2")
                    y2 = sp.tile([128, D], F32, tag="y2")
                    nc.vector.scalar_tensor_tensor(y2[:], pf[:], 0.0, ln1fw[w][:],
                                                   ALU.add, ALU.add,
                                                   accum_out=st2[:, 0:1])
                    sq2 = sp.tile([128, D], F32, tag="sq2")
                    nc.vector.scalar_tensor_tensor(sq2[:], y2[:], 0.0, y2[:],
                                                   ALU.add, ALU.mult,
                                                   accum_out=st2[:, 1:2])
                    nc.scalar.mul(st2[:, 2:3], st2[:, 0:1], 1.0 / D)
                    nc.vector.tensor_mul(st2[:, 3:4], st2[:, 2:3], st2[:, 2:3])
                    nc.vector.scalar_tensor_tensor(st2[:, 4:5], st2[:, 1:2], 1.0 / D,
                                                   st2[:, 3:4], ALU.mult,
                                                   ALU.subtract)
                    nc.scalar.activation(st2[:, 5:6], st2[:, 4:5], AF.Sqrt, bias=eps_t[:])
                    nc.vector.reciprocal(st2[:, 6:7], st2[:, 5:6])
                    nc.vector.scalar_tensor_tensor(st2[:, 7:8], st2[:, 2:3], -1.0,
                                                   st2[:, 6:7], ALU.mult, ALU.mult)
                    xh2 = sp.tile([128, D], F32, tag="xh2")
                    nc.scalar.activation(xh2[:], y2[:], AF.Identity,
                                         bias=st2[:, 7:8], scale=st2[:, 6:7])
                    yo = sp.tile([128, D], F32, tag="yo")
                    nc.vector.tensor_mul(xh2[:], xh2[:], g2b[:])
                    nc.vector.tensor_add(yo[:], xh2[:], bb2[:])
                    nc.sync.dma_start(ov[2 * p + w], yo[:])

    nc.compile()
    return nc


def _pack(wT, kc):
    """[kc*128, N] -> [128, kc*N] with partition p, block k = wT[k*128+p]."""
    n = wT.shape[1]
    return np.ascontiguousarray(
        wT.reshape(kc, 128, n).transpose(1, 0, 2).reshape(128, kc * n))


_CACHE = {}


def _get_nc(n_pairs=PAIRS):
    if n_pairs not in _CACHE:
        _CACHE[n_pairs] = _build_nc(n_pairs)
    return _CACHE[n_pairs]


def _prep_inputs(src, in_proj_w, in_proj_b, out_w, out_b, ln1_g, ln1_b,
                 w1, b1, w2, b2, ln2_g, ln2_b, n_pairs=PAIRS):
    src = np.asarray(src, np.float32)
    scale = 1.0 / np.sqrt(HD)

    wqkT = np.asarray(in_proj_w[:2 * D], np.float32).T.copy()   # [512, 1024]
    wqkT[:, :D] *= scale
    bqk = np.asarray(in_proj_b[:2 * D], np.float32).copy()
    bqk[:D] *= scale

    common = {
        "wqk": _pack(wqkT.astype(BF16), KC),
        "wv": _pack(np.asarray(in_proj_w[2 * D:], np.float32).T.astype(BF16), KC),
        "wo": _pack(np.asarray(out_w, np.float32).T.astype(BF16), KC),
        "w1t": _pack(np.asarray(w1, np.float32).T.astype(BF16), KC),
        "w2t": _pack(np.asarray(w2, np.float32).T.astype(BF16), FC),
        "qkb": np.ascontiguousarray(bqk.reshape(8, 128).T),
        "b1t": np.ascontiguousarray(np.asarray(b1, np.float32).reshape(FC, 128).T),
        "vbr": np.asarray(in_proj_b[2 * D:], np.float32).astype(BF16)[None, :],
        "obr": np.asarray(out_b, np.float32).astype(BF16)[None, :],
        "b2r": np.asarray(b2, np.float32).astype(BF16)[None, :],
        "g1b": np.ascontiguousarray(np.broadcast_to(np.asarray(ln1_g, np.float32),
                                                    (128, D))),
        "bb1": np.ascontiguousarray(np.broadcast_to(np.asarray(ln1_b, np.float32),
                                                    (128, D))),
        "g2b": np.ascontiguousarray(np.broadcast_to(np.asarray(ln2_g, np.float32),
                                                    (128, D))),
        "bb2": np.ascontiguousarray(np.broadcast_to(np.asarray(ln2_b, np.float32),
                                                    (128, D))),
        "ident": np.eye(128, dtype=BF16),
        "ones1": np.ones((1, 128), BF16),
    }

    wins = src.reshape(NW_TOT, W, D)
    wpc = n_pairs * 2
    in_maps = []
    for c in range(N_CORES):
        m = dict(common)
        m["x"] = np.ascontiguousarray(
            wins[c * wpc:(c + 1) * wpc].reshape(wpc * W, D))
        in_maps.append(m)
    return in_maps


def kernel(src, in_proj_w, in_proj_b, out_w, out_b, ln1_g, ln1_b,
           w1, b1, w2, b2, ln2_g, ln2_b):
    nc = _get_nc()
    in_maps = _prep_inputs(src, in_proj_w, in_proj_b, out_w, out_b, ln1_g,
                           ln1_b, w1, b1, w2, b2, ln2_g, ln2_b)
    res = run_bass_kernel_spmd(nc, in_maps, list(range(N_CORES)))
    out = np.concatenate([res.results[c]["out"] for c in range(N_CORES)], axis=0)
    return np.ascontiguousarray(out.reshape(B, S, D)).astype(np.float32)


# revision 7
# speedup vs baseline: 1.1060x; 1.1060x over previous
"""Local-window attention encoder layer on 8 Trainium2 cores.

Problem: B=4, S=8192, D=512, window W=128, H=8 heads (HD=64), FF dim 2048.
Sharding: [B*nW]=256 independent windows split 32/core across 8 cores.

Per-core device kernel (bf16 matmuls, f32 residual/softmax/LN):
  windows processed in pairs (256 tokens on the matmul free axis).
  qkT (e-major) via W-stationary matmuls; v kept token-major; scores per
  head from qkT slices; softmax over the free axis with ACT-accumulated
  row sums; probs normalized then PE-transposed for attnT; out-proj /
  FF1 / FF2 as K-chunked accumulating matmuls with rank-1 bias matmuls.
"""

import numpy as np
import ml_dtypes

import concourse.bass as bass
import concourse.tile as tile
from concourse import bacc, mybir
from concourse.bass_utils import run_bass_kernel_spmd

BF16 = ml_dtypes.bfloat16
F32 = mybir.dt.float32
BF = mybir.dt.bfloat16
AF = mybir.ActivationFunctionType
ALU = mybir.AluOpType

D = 512
H = 8
W = 128
HD = 64
FF = 2048
EPS = 1e-5
N_CORES = 8
B, S = 4, 8192
NW_TOT = (B * S) // W          # 256 windows
WPC = NW_TOT // N_CORES        # 32 windows per core
PAIRS = WPC // 2               # 16 pairs per core
KC = D // 128                  # 4 contraction chunks of 128
FC = FF // 128                 # 16 ff chunks


def _build_nc(n_pairs=PAIRS):
    nc = bacc.Bacc("TRN2", target_bir_lowering=False, debug=False,
                   num_devices=N_CORES)
    n_tok = n_pairs * 2 * W

    x_d = nc.dram_tensor("x", [n_tok, D], F32, kind="ExternalInput").ap()
    out_d = nc.dram_tensor("out", [n_tok, D], F32, kind="ExternalOutput").ap()
    wqk_d = nc.dram_tensor("wqk", [128, KC * 1024], BF, kind="ExternalInput").ap()
    wv_d = nc.dram_tensor("wv", [128, KC * D], BF, kind="ExternalInput").ap()
    wo_d = nc.dram_tensor("wo", [128, KC * D], BF, kind="ExternalInput").ap()
    w1_d = nc.dram_tensor("w1t", [128, KC * FF], BF, kind="ExternalInput").ap()
    w2_d = nc.dram_tensor("w2t", [128, FC * D], BF, kind="ExternalInput").ap()
    qkb_d = nc.dram_tensor("qkb", [128, 8], F32, kind="ExternalInput").ap()
    b1_d = nc.dram_tensor("b1t", [128, FC], F32, kind="ExternalInput").ap()
    vb_d = nc.dram_tensor("vbr", [1, D], BF, kind="ExternalInput").ap()
    ob_d = nc.dram_tensor("obr", [1, D], BF, kind="ExternalInput").ap()
    b2_d = nc.dram_tensor("b2r", [1, D], BF, kind="ExternalInput").ap()
    g1_d = nc.dram_tensor("g1b", [128, D], F32, kind="ExternalInput").ap()
    bb1_d = nc.dram_tensor("bb1", [128, D], F32, kind="ExternalInput").ap()
    g2_d = nc.dram_tensor("g2b", [128, D], F32, kind="ExternalInput").ap()
    bb2_d = nc.dram_tensor("bb2", [128, D], F32, kind="ExternalInput").ap()
    id_d = nc.dram_tensor("ident", [128, 128], BF, kind="ExternalInput").ap()
    on_d = nc.dram_tensor("ones1", [1, 128], BF, kind="ExternalInput").ap()

    xv = x_d.rearrange("(w p) d -> w p d", p=W)
    ov = out_d.rearrange("(w p) d -> w p d", p=W)

    with tile.TileContext(nc) as tc:
        with (
            tc.tile_pool(name="const", bufs=1) as cp,
            tc.tile_pool(name="stream", bufs=2) as sp,
            tc.tile_pool(name="deep", bufs=4) as sp4,
            tc.tile_pool(name="ps", bufs=3, space="PSUM") as pp,
            tc.tile_pool(name="pst", bufs=2, space="PSUM") as ppt,
            tc.tile_pool(name="sc", bufs=3, space="PSUM") as scp,
        ):
            # ---- resident constants ----
            wqk = cp.tile([128, KC, 1024], BF); nc.sync.dma_start(wqk[:], wqk_d[:])
            wv = cp.tile([128, KC, D], BF); nc.sync.dma_start(wv[:], wv_d[:])
            wo = cp.tile([128, KC, D], BF); nc.sync.dma_start(wo[:], wo_d[:])
            w1t = cp.tile([128, KC, FF], BF); nc.sync.dma_start(w1t[:], w1_d[:])
            w2t = cp.tile([128, FC, D], BF); nc.sync.dma_start(w2t[:], w2_d[:])
            qkb = cp.tile([128, 8], F32); nc.sync.dma_start(qkb[:], qkb_d[:])
            b1t = cp.tile([128, FC], F32); nc.sync.dma_start(b1t[:], b1_d[:])
            vbr = cp.tile([1, D], BF); nc.sync.dma_start(vbr[:], vb_d[:])
            obr = cp.tile([1, D], BF); nc.sync.dma_start(obr[:], ob_d[:])
            b2r = cp.tile([1, D], BF); nc.sync.dma_start(b2r[:], b2_d[:])
            g1b = cp.tile([128, D], F32); nc.sync.dma_start(g1b[:], g1_d[:])
            bb1 = cp.tile([128, D], F32); nc.sync.dma_start(bb1[:], bb1_d[:])
            g2b = cp.tile([128, D], F32); nc.sync.dma_start(g2b[:], g2_d[:])
            bb2 = cp.tile([128, D], F32); nc.sync.dma_start(bb2[:], bb2_d[:])
            ident = cp.tile([128, 128], BF); nc.sync.dma_start(ident[:], id_d[:])
            eps_t = cp.tile([128, 1], F32); nc.vector.memset(eps_t[:], EPS)
            ones1 = cp.tile([1, 128], BF); nc.sync.dma_start(ones1[:], on_d[:])

            def layernorm(y, g, b, out_f32, out_bf16=None):
                """y [128,D] f32 (already residual-added, row-sum in st col0).
                Returns nothing; writes out tiles. st supplied by caller."""
                pass  # inlined below instead

            for p in range(n_pairs):
                xw = []
                for w in range(2):
                    x = sp4.tile([W, D], F32, tag="x")
                    nc.sync.dma_start(x[:], xv[2 * p + w])
                    xw.append(x)

                # transpose x (bf16) -> xT_pair [128, KC, 256]
                xtp = sp.tile([128, KC, 2 * W], BF, tag="xtp")
                for w in range(2):
                    xb = sp.tile([W, D], BF, tag="xb")
                    nc.vector.tensor_copy(xb[:], xw[w][:])
                    tp = ppt.tile([128, KC, 128], BF, tag="pst")
                    for k in range(KC):
                        nc.tensor.transpose(tp[:, k, :], xb[:, k * 128:(k + 1) * 128],
                                            ident[:])
                    nc.vector.tensor_copy(xtp[:, :, w * W:(w + 1) * W], tp[:])

                # qkT: [e-chunk 8][128, 256]
                qkt = sp.tile([128, 8, 2 * W], BF, tag="qkt")
                for m in range(8):
                    pq = pp.tile([128, 2 * W], F32, tag="ps")
                    for k in range(KC):
                        nc.tensor.matmul(
                            pq[:], wqk[:, k, m * 128:(m + 1) * 128], xtp[:, k, :],
                            start=(k == 0), stop=(k == KC - 1))
                    nc.scalar.activation(qkt[:, m, :], pq[:], AF.Identity,
                                         bias=qkb[:, m:m + 1])

                # v token-major per window: [128 tok, 512]
                vw = []
                for w in range(2):
                    pv = pp.tile([128, D], F32, tag="ps")
                    for k in range(KC):
                        nc.tensor.matmul(
                            pv[:], xtp[:, k, w * W:(w + 1) * W], wv[:, k, :],
                            start=(k == 0), stop=False)
                    nc.tensor.matmul(pv[:], ones1[:], vbr[:], start=False, stop=True)
                    v = sp4.tile([128, D], BF, tag="v")
                    nc.vector.tensor_copy(v[:], pv[:])
                    vw.append(v)

                ln1fw = []
                for w in range(2):
                    # ---- attention ----
                    sums = sp.tile([128, 8], F32, tag="sums")
                    probs = []
                    for h in range(8):
                        psc = scp.tile([128, 128], F32, tag="sc")
                        pb = (h % 2) * 64
                        lq = qkt[pb:pb + 64, h // 2, w * W:(w + 1) * W]
                        lk = qkt[pb:pb + 64, 4 + h // 2, w * W:(w + 1) * W]
                        nc.tensor.matmul(psc[:], lq, lk, start=True, stop=True,
                                         tile_position=(pb, 0))
                        pr = sp.tile([128, 128], BF, tag=f"pr{h}")
                        nc.scalar.activation(pr[:], psc[:], AF.Exp,
                                             accum_out=sums[:, h:h + 1])
                        probs.append(pr)
                    recip = sp.tile([128, 8], F32, tag="recip")
                    for h in range(8):
                        nc.vector.reciprocal(recip[:, h:h + 1], sums[:, h:h + 1])
                    pat = pp.tile([128, D], F32, tag="ps")
                    for h in range(8):
                        pn = sp.tile([128, 128], BF, tag=f"pn{h%4}")
                        nc.vector.tensor_scalar_mul(pn[:], probs[h][:],
                                                    recip[:, h:h + 1])
                        ptp = scp.tile([128, 128], BF, tag="sc")
                        nc.tensor.transpose(ptp[:], pn[:], ident[:])
                        pts = sp.tile([128, 128], BF, tag=f"pt{h%4}")
                        nc.vector.tensor_copy(pts[:], ptp[:])
                        pb = (h % 2) * 64
                        nc.tensor.matmul(
                            pat[pb:pb + 64, (h // 2) * 128:(h // 2 + 1) * 128],
                            vw[w][:, h * HD:(h + 1) * HD], pts[:],
                            start=True, stop=True, tile_position=(0, pb))
                    ats = sp.tile([128, D], BF, tag="ats")
                    nc.vector.tensor_copy(ats[:], pat[:])

                    # out projection
                    pao = pp.tile([128, D], F32, tag="ps")
                    for k in range(KC):
                        nc.tensor.matmul(pao[:], ats[:, k * 128:(k + 1) * 128],
                                         wo[:, k, :], start=(k == 0), stop=False)
                    nc.tensor.matmul(pao[:], ones1[:], obr[:], start=False, stop=True)

                    # residual 1 + LN1
                    st = sp.tile([128, 8], F32, tag="st")
                    y1 = sp.tile([128, D], F32, tag="y1")
                    nc.vector.scalar_tensor_tensor(y1[:], pao[:], 0.0, xw[w][:],
                                                   ALU.add, ALU.add,
                                                   accum_out=st[:, 0:1])
                    sq = sp.tile([128, D], F32, tag="sq")
                    nc.vector.scalar_tensor_tensor(sq[:], y1[:], 0.0, y1[:],
                                                   ALU.add, ALU.mult,
                                                   accum_out=st[:, 1:2])
                    nc.scalar.mul(st[:, 2:3], st[:, 0:1], 1.0 / D)
                    nc.vector.tensor_mul(st[:, 3:4], st[:, 2:3], st[:, 2:3])
                    nc.vector.scalar_tensor_tensor(st[:, 4:5], st[:, 1:2], 1.0 / D,
                                                   st[:, 3:4], ALU.mult, ALU.subtract)
                    nc.scalar.activation(st[:, 5:6], st[:, 4:5], AF.Sqrt, bias=eps_t[:])
                    nc.vector.reciprocal(st[:, 6:7], st[:, 5:6])
                    nc.vector.scalar_tensor_tensor(st[:, 7:8], st[:, 2:3], -1.0,
                                                   st[:, 6:7], ALU.mult, ALU.mult)
                    xh = sp.tile([128, D], F32, tag="xh")
                    nc.scalar.activation(xh[:], y1[:], AF.Identity,
                                         bias=st[:, 7:8], scale=st[:, 6:7])
                    ln1f = sp4.tile([128, D], F32, tag="ln1f")
                    nc.vector.tensor_mul(xh[:], xh[:], g1b[:])
                    nc.vector.tensor_add(ln1f[:], xh[:], bb1[:])
                    ln1fw.append(ln1f)

                # transpose ln1 (both windows) -> lnT [128, KC, 256]
                lnt = sp.tile([128, KC, 2 * W], BF, tag="lnt")
                for w in range(2):
                    lb = sp.tile([W, D], BF, tag="lb")
                    nc.vector.tensor_copy(lb[:], ln1fw[w][:])
                    tp2 = ppt.tile([128, KC, 128], BF, tag="pst")
                    for k in range(KC):
                        nc.tensor.transpose(tp2[:, k, :], lb[:, k * 128:(k + 1) * 128],
                                            ident[:])
                    nc.vector.tensor_copy(lnt[:, :, w * W:(w + 1) * W], tp2[:])

                # FF1: h1T [f-chunk 16][128, 256]
                h1 = sp.tile([128, FC, 2 * W], BF, tag="h1")
                for m in range(FC):
                    ph = pp.tile([128, 2 * W], F32, tag="ps")
                    for k in range(KC):
                        nc.tensor.matmul(
                            ph[:], w1t[:, k, m * 128:(m + 1) * 128], lnt[:, k, :],
                            start=(k == 0), stop=(k == KC - 1))
                    nc.scalar.activation(h1[:, m, :], ph[:], AF.Relu,
                                         bias=b1t[:, m:m + 1])

                # FF2 + residual2 + LN2 per window
                for w in range(2):
                    pf = pp.tile([128, D], F32, tag="ps")
                    for m in range(FC):
                        nc.tensor.matmul(pf[:], h1[:, m, w * W:(w + 1) * W],
                                         w2t[:, m, :], start=(m == 0), stop=False)
                    nc.tensor.matmul(pf[:], ones1[:], b2r[:], start=False, stop=True)

                    st2 = sp.tile([128, 8], F32, tag="st2")
                    y2 = sp.tile([128, D], F32, tag="y2")
                    nc.vector.scalar_tensor_tensor(y2[:], pf[:], 0.0, ln1fw[w][:],
                                                   ALU.add, ALU.add,
                                                   accum_out=st2[:, 0:1])
                    sq2 = sp.tile([128, D], F32, tag="sq2")
                    nc.vector.scalar_tensor_tensor(sq2[:], y2[:], 0.0, y2[:],
                                                   ALU.add, ALU.mult,
                                                   accum_out=st2[:, 1:2])
                    nc.scalar.mul(st2[:, 2:3], st2[:, 0:1], 1.0 / D)
                    nc.vector.tensor_mul(st2[:, 3:4], st2[:, 2:3], st2[:, 2:3])
                    nc.vector.scalar_tensor_tensor(st2[:, 4:5], st2[:, 1:2], 1.0 / D,
                                                   st2[:, 3:4], ALU.mult,
                                                   ALU.subtract)
                    nc.scalar.activation(st2[:, 5:6], st2[:, 4:5], AF.Sqrt, bias=eps_t[:])
                    nc.vector.reciprocal(st2[:, 6:7], st2[:, 5:6])
                    nc.vector.scalar_tensor_tensor(st2[:, 7:8], st2[:, 2:3], -1.0,
                                                   st2[:, 6:7], ALU.mult, ALU.mult)
                    xh2 = sp.tile([128, D], F32, tag="xh2")
                    nc.scalar.activation(xh2[:], y2[:], AF.Identity,
                                         bias=st2[:, 7:8], scale=st2[:, 6:7])
                    yo = sp.tile([128, D], F32, tag="yo")
                    nc.vector.tensor_mul(xh2[:], xh2[:], g2b[:])
                    nc.vector.tensor_add(yo[:], xh2[:], bb2[:])
                    nc.sync.dma_start(ov[2 * p + w], yo[:])

    nc.compile()
    return nc


def _pack(wT, kc):
    """[kc*128, N] -> [128, kc*N] with partition p, block k = wT[k*128+p]."""
    n = wT.shape[1]
    return np.ascontiguousarray(
        wT.reshape(kc, 128, n).transpose(1, 0, 2).reshape(128, kc * n))


_CACHE = {}


def _get_nc(n_pairs=PAIRS):
    if n_pairs not in _CACHE:
        _CACHE[n_pairs] = _build_nc(n_pairs)
    return _CACHE[n_pairs]


def _prep_inputs(src, in_proj_w, in_proj_b, out_w, out_b, ln1_g, ln1_b,
                 w1, b1, w2, b2, ln2_g, ln2_b, n_pairs=PAIRS):
    src = np.asarray(src, np.float32)
    scale = 1.0 / np.sqrt(HD)

    wqkT = np.asarray(in_proj_w[:2 * D], np.float32).T.copy()   # [512, 1024]
    wqkT[:, :D] *= scale
    bqk = np.asarray(in_proj_b[:2 * D], np.float32).copy()
    bqk[:D] *= scale

    common = {
        "wqk": _pack(wqkT.astype(BF16), KC),
        "wv": _pack(np.asarray(in_proj_w[2 * D:], np.float32).T.astype(BF16), KC),
        "wo": _pack(np.asarray(out_w, np.float32).T.astype(BF16), KC),
        "w1t": _pack(np.asarray(w1, np.float32).T.astype(BF16), KC),
        "w2t": _pack(np.asarray(w2, np.float32).T.astype(BF16), FC),
        "qkb": np.ascontiguousarray(bqk.reshape(8, 128).T),
        "b1t": np.ascontiguousarray(np.asarray(b1, np.float32).reshape(FC, 128).T),
        "vbr": np.asarray(in_proj_b[2 * D:], np.float32).astype(BF16)[None, :],
        "obr": np.asarray(out_b, np.float32).astype(BF16)[None, :],
        "b2r": np.asarray(b2, np.float32).astype(BF16)[None, :],
        "g1b": np.ascontiguousarray(np.broadcast_to(np.asarray(ln1_g, np.float32),
                                                    (128, D))),
        "bb1": np.ascontiguousarray(np.broadcast_to(np.asarray(ln1_b, np.float32),
                                                    (128, D))),
        "g2b": np.ascontiguousarray(np.broadcast_to(np.asarray(ln2_g, np.float32),
                                                    (128, D))),
        "bb2": np.ascontiguousarray(np.broadcast_to(np.asarray(ln2_b, np.float32),
                                                    (128, D))),
        "ident": np.eye(128, dtype=BF16),
        "ones1": np.ones((1, 128), BF16),
    }

    wins = src.reshape(NW_TOT, W, D)
    wpc = n_pairs * 2
    in_maps = []
    for c in range(N_CORES):
        m = dict(common)
        m["x"] = np.ascontiguousarray(
            wins[c * wpc:(c + 1) * wpc].reshape(wpc * W, D))
        in_maps.append(m)
    return in_maps


def kernel(src, in_proj_w, in_proj_b, out_w, out_b, ln1_g, ln1_b,
           w1, b1, w2, b2, ln2_g, ln2_b):
    nc = _get_nc()
    in_maps = _prep_inputs(src, in_proj_w, in_proj_b, out_w, out_b, ln1_g,
                           ln1_b, w1, b1, w2, b2, ln2_g, ln2_b)
    res = run_bass_kernel_spmd(nc, in_maps, list(range(N_CORES)))
    out = np.concatenate([res.results[c]["out"] for c in range(N_CORES)], axis=0)
    return np.ascontiguousarray(out.reshape(B, S, D)).astype(np.float32)
